# revision 1
# baseline (speedup 1.0000x reference)
"""Trainium2 Bass kernel for nn_Attn_66297115181215 (sparse_attention).

Reference computation (B=2, N=8192, C=256, H=8, Dh=C):
    qh/kh/vh = heads(emb @ W{q,k,v})            [B,H,N,Dh]
    attn = einsum("bhnd,bhne->bhde", qh, kh)    [B,H,Dh,Dh]
    attn = instance_norm(attn); attn = softmax(attn, axis=3)
    ctx  = einsum("bhde,bhne->bhdn", attn, vh)  [B,H,Dh,N]
    out  = ctx.transpose(0,3,2,1).reshape(B,N,C*H) @ Wo

Algebraic collapse used here: the sequence dim N only enters through
G_b = emb_b^T @ emb_b  [C,C], because
    attn_h = Wq_h^T G Wk_h
and the output is
    out_b = emb_b @ P_b,  P_b = sum_h Wv_h @ M_h,  M_h = S_h^T Wo'_h
where S_h = softmax(instnorm(attn_h)) (the softmax denominator is folded
into Wo'_h) and Wo_h = Wo[d*H+h] rows.  ~16x fewer FLOPs than
materializing Q/K/V.

Distribution (8 cores, no collectives — a 256KB AllReduce measured
~60us here, dwarfing the kernel):
  core c: b = c//4, j = c%4.  Every core redundantly computes G_b (full
  emb_b read) and the full 8-head chain (small [256,256] matmuls), then
  computes its own N/4 slice of out_b = emb_b @ P_b, transposed
  ([C, CHUNK], host transposes back).

Precision: G and the head chain run in bf16 (fp32 accumulate in PSUM);
the final out projection runs float32r (fp22 multiply).  End-to-end
relative error ~5e-3 (numpy-simulated, hardware-verified).
"""

import os
import sys

sys.path.insert(0, "/opt/trn_rl_repo")

import numpy as np

import concourse.bacc as bacc
import concourse.mybir as mybir
import concourse.tile as tile
from concourse.tile import add_dep_helper
from concourse.bass_utils import run_bass_kernel_spmd

B, N, C, H = 2, 8192, 256, 8
EPS = 1e-5
NCORES = 8
CHUNK = N * B // NCORES  # 2048 rows of out per core
NQ = 4                   # emb arrives in 4 chunk DMAs (16KB lines)
TQ = N // 128 // NQ      # 16 column-blocks of 256 per chunk

F32 = mybir.dt.float32
F32R = mybir.dt.float32r
BF16 = mybir.dt.bfloat16
AF = mybir.ActivationFunctionType
ALU = mybir.AluOpType
AX = mybir.AxisListType


def build_kernel():
    nc = bacc.Bacc("TRN2", target_bir_lowering=False, debug=False,
                   num_devices=NCORES)

    # emb arrives p-major: partition p holds 64 contiguous rows
    # (16KB DRAM lines per quarter-DMA). G is row-order agnostic.
    emb = nc.dram_tensor("emb", [N, C], F32, kind="ExternalInput")
    # host-packed p-major layouts (contiguous 16KB DRAM lines per partition):
    # wqk [128, 8192]: wq half0 | wq half1 | wk half0 | wk half1
    wqk = nc.dram_tensor("wqk", [128, 4 * C * H], F32, kind="ExternalInput")
    # wvt/wos [128, 4096]: col t*256+c = row t*128+p of Wv^T / WoS
    wvt = nc.dram_tensor("wvt", [128, 16 * C], F32, kind="ExternalInput")
    wos = nc.dram_tensor("wos", [128, 16 * C], F32, kind="ExternalInput")
    # embt [128, 4096]: col i*2048+n = embT row i*128+p
    embt = nc.dram_tensor("embt", [128, 16 * C], F32R, kind="ExternalInput")
    outt = nc.dram_tensor("outt", [C, CHUNK], F32, kind="ExternalOutput")

    with tile.TileContext(nc) as tc:
        with (
            tc.tile_pool(name="stage", bufs=3) as stage,
            tc.tile_pool(name="wbf", bufs=1) as wbf,
            tc.tile_pool(name="persist", bufs=1) as persist,
            tc.tile_pool(name="abuf", bufs=16) as abuf,
            tc.tile_pool(name="ebuf", bufs=16) as ebuf,
            tc.tile_pool(name="chain", bufs=6) as chain,
        ):
            # ---------- loads + bf16 casts (emb first: G gates the chain) ----
            # graded piece sizes (in 256-col j-blocks): big first, tiny last,
            # so the G matmuls gated on the final DMA are only ~2 blocks.
            PIECES = [16, 14, 12, 8, 6, 4, 2, 2]
            assert sum(PIECES) == N // 128
            emb_bf = [persist.tile([128, nb * C], BF16, name=f"ebf{q}")
                      for q, nb in enumerate(PIECES)]
            emb_r = emb.rearrange("(p t) c -> p t c", p=128)  # t: 64 blocks
            off = 0
            for q, nb in enumerate(PIECES):
                s = stage.tile([128, nb * C], F32, name="est", tag="est",
                               bufs=2)
                nc.sync.dma_start(
                    s[:].rearrange("p (t c) -> p t c", c=C),
                    emb_r[:, off:off + nb, :])
                off += nb
                if q % 2 == 0:
                    nc.scalar.copy(emb_bf[q][:], s[:])
                else:
                    nc.vector.tensor_copy(emb_bf[q][:], s[:])

            # wqk packed: wq|wk halves; casts on idle GpSimd
            qk_bf = wbf.tile([128, 4 * C * H], BF16, name="qkb")
            wq_bf = [qk_bf[:, i * C * H:(i + 1) * C * H] for i in range(2)]
            wk_bf = [qk_bf[:, (2 + i) * C * H:(3 + i) * C * H] for i in range(2)]
            for i in range(4):
                s = stage.tile([128, C * H], F32, name="wst", tag="wst")
                nc.sync.dma_start(s[:], wqk[:, i * C * H:(i + 1) * C * H])
                nc.vector.tensor_copy(
                    qk_bf[:, i * C * H:(i + 1) * C * H], s[:])

            # wvt/wos (host-packed [128, 4096]) -> bf16 via GpSimd
            wvt_bf = wbf.tile([128, 16 * C], BF16, name="wvtb")
            wos_bf = wbf.tile([128, 16 * C], BF16, name="wosb")
            for srct, dst in ((wvt, wvt_bf), (wos, wos_bf)):
                for hh in range(2):
                    s = stage.tile([128, 8 * C], F32, name="wst", tag="wst")
                    nc.sync.dma_start(s[:], srct[:, hh * 8 * C:(hh + 1) * 8 * C])
                    nc.vector.tensor_copy(dst[:, hh * 8 * C:(hh + 1) * 8 * C], s[:])

            embt_full = persist.tile([128, 16 * C], F32R, name="etf")
            for i in range(2):
                nc.sync.dma_start(embt_full[:, i * 8 * C:(i + 1) * 8 * C],
                                  embt[:, i * 8 * C:(i + 1) * 8 * C])
            embt_sb = [embt_full[:, i * CHUNK:(i + 1) * CHUNK] for i in range(2)]

            ones = persist.tile([128, 128], F32, name="ones")
            nc.vector.memset(ones[:], 1.0)
            epst = persist.tile([128, 1], F32, name="epst")
            nc.vector.memset(epst[:], EPS)

            # ---------- G = emb^T @ emb (bf16, fp32 accumulate) ----------
            g_bf = [persist.tile([128, C], BF16, name=f"gbf{i}") for i in range(2)]
            with tc.tile_pool(name="psg", bufs=1, space="PSUM") as psg:
                g_ps = [psg.tile([128, C], F32, name=f"g{i}") for i in range(2)]
                nq = len(PIECES)
                for q, nb in enumerate(PIECES):
                    for j in range(nb):
                        blk = emb_bf[q][:, j * C:(j + 1) * C]
                        for ch in range(2):
                            nc.tensor.matmul(
                                g_ps[ch][:],
                                emb_bf[q][:, j * C + ch * 128:j * C + (ch + 1) * 128],
                                blk,
                                start=(q == 0 and j == 0),
                                stop=(q == nq - 1 and j == nb - 1))
                for ch in range(2):
                    nc.vector.tensor_copy(g_bf[ch][:], g_ps[ch][:])

            pswork_cm = tc.tile_pool(name="pswork", bufs=1, space="PSUM")
            pswork = pswork_cm.__enter__()
            p_ps = [pswork.tile([128, C], F32, name=f"p{i}", tag=f"p{i}")[:]
                    for i in range(2)]

            # ---------- U_all = G @ Wk (all heads, free=512 chunks) ----------
            u_bf = [persist.tile([128, C * H], BF16, name=f"ubf{i}")
                    for i in range(2)]
            for f in range(4):
                fs = slice(f * 512, (f + 1) * 512)
                for mh in range(2):
                    u_ps = pswork.tile([128, 512], F32, name="ups", tag="ups",
                                       bufs=2)
                    for kc in range(2):
                        nc.tensor.matmul(
                            u_ps[:],
                            g_bf[kc][:, mh * 128:(mh + 1) * 128],
                            qk_bf[:, kc * C * H + f * 512:kc * C * H + (f + 1) * 512],
                            start=(kc == 0), stop=(kc == 1))
                    nc.scalar.copy(u_bf[mh][:, fs], u_ps[:])

            # ---------- A_h = Wq_h^T @ U_h ; stats ----------
            inv_n2 = 1.0 / float(C * C)
            a_sb = []      # per (h, dh): [128, C] fp32
            statc = persist.tile([128, 4 * H], F32, name="statc")
            for h in range(H):
                a_h = []
                for dh in range(2):
                    a_ps = pswork.tile([128, C], F32, name="aps", tag="work",
                                       bufs=4)
                    for kc in range(2):
                        nc.tensor.matmul(
                            a_ps[:],
                            qk_bf[:, (2 + kc) * C * H + h * C + dh * 128:(2 + kc) * C * H + h * C + (dh + 1) * 128],
                            u_bf[kc][:, h * C:(h + 1) * C],
                            start=(kc == 0), stop=(kc == 1))
                    at = abuf.tile([128, C], F32, name="at", tag="at")
                    nc.vector.tensor_scalar(
                        at[:], a_ps[:], 1.0, 0.0, ALU.mult, ALU.add,
                        accum_out=statc[:, 4 * h + dh:4 * h + dh + 1])
                    sqscr = chain.tile([128, C], F32, name="sqscr", tag="sqscr")
                    nc.scalar.activation(
                        sqscr[:], a_ps[:], AF.Square,
                        accum_out=statc[:, 4 * h + 2 + dh:4 * h + 3 + dh])
                    a_h.append(at)
                a_sb.append(a_h)

            # ---------- batched instance-norm stats ----------
            st_ps = pswork.tile([128, 4 * H], F32, name="stps", tag="work",
                                bufs=4)
            nc.tensor.matmul(st_ps[:], ones[:], statc[:], start=True, stop=True)
            st_sb = chain.tile([128, 4 * H], F32, name="st_sb", tag="st_sb")
            nc.vector.tensor_copy(st_sb[:], st_ps[:])
            s1t = chain.tile([128, H], F32, name="s1t", tag="s1t")
            nc.vector.tensor_add(s1t[:], st_sb[:, 0::4], st_sb[:, 1::4])
            s2t = chain.tile([128, H], F32, name="s2t", tag="s2t")
            nc.vector.tensor_add(s2t[:], st_sb[:, 2::4], st_sb[:, 3::4])
            mu = chain.tile([128, H], F32, name="mu", tag="mu")
            nc.vector.tensor_scalar_mul(mu[:], s1t[:], inv_n2)
            e2 = chain.tile([128, H], F32, name="e2", tag="e2")
            nc.vector.tensor_scalar_mul(e2[:], s2t[:], inv_n2)
            mu2 = chain.tile([128, H], F32, name="mu2", tag="mu2")
            nc.vector.tensor_mul(mu2[:], mu[:], mu[:])
            var = chain.tile([128, H], F32, name="var", tag="var")
            nc.vector.tensor_sub(var[:], e2[:], mu2[:])
            sd = chain.tile([128, H], F32, name="sd", tag="sd")
            nc.scalar.activation(sd[:], var[:], AF.Sqrt, bias=epst[:])
            rinv = chain.tile([128, H], F32, name="rinv", tag="rinv")
            nc.vector.reciprocal(rinv[:], sd[:])

            # ---------- softmax numerators (one ACT table load) ----------
            esum = persist.tile([128, 2 * H], F32, name="esum")
            rec = persist.tile([128, 2 * H], F32, name="rec")
            e_bf = []
            for h in range(H):
                e_h = []
                for dh in range(2):
                    et = ebuf.tile([128, C], BF16, name="et", tag="et")
                    nc.scalar.activation(
                        et[:], a_sb[h][dh][:], AF.Exp,
                        scale=rinv[:, h:h + 1])
                    nc.vector.tensor_reduce(
                        esum[:, 2 * h + dh:2 * h + dh + 1], et[:],
                        AX.X, ALU.add)
                    e_h.append(et)
                e_bf.append(e_h)
                nc.vector.reciprocal(rec[:, 2 * h:2 * h + 2],
                                     esum[:, 2 * h:2 * h + 2])

            # ---------- M_h = E^T @ (Wo_h rows * rec) ; P += Wv_h @ M_h ----
            for h in range(H):
                woh = [chain.tile([128, C], BF16, name="woh", tag="woh")
                       for _ in range(2)]
                for dh in range(2):
                    nc.vector.tensor_scalar_mul(
                        woh[dh][:],
                        wos_bf[:, (2 * h + dh) * C:(2 * h + dh + 1) * C],
                        rec[:, 2 * h + dh:2 * h + dh + 1])
                m_bf = [chain.tile([128, C], BF16, name="mbf", tag="mbf")
                        for _ in range(2)]
                for eh in range(2):
                    m_ps = pswork.tile([128, C], F32, name="mps", tag="work",
                                       bufs=4)
                    for kc in range(2):
                        nc.tensor.matmul(
                            m_ps[:],
                            e_bf[h][kc][:, eh * 128:(eh + 1) * 128],
                            woh[kc][:],
                            start=(kc == 0), stop=(kc == 1))
                    nc.vector.tensor_copy(m_bf[eh][:], m_ps[:])
                for ch in range(2):
                    for kc in range(2):
                        nc.tensor.matmul(
                            p_ps[ch],
                            wvt_bf[:, (2 * h + kc) * C + ch * 128:
                                   (2 * h + kc) * C + (ch + 1) * 128],
                            m_bf[kc][:],
                            start=(h == 0 and kc == 0),
                            stop=(h == H - 1 and kc == 1))

            p_sb = [persist.tile([128, C], F32R, name=f"psb{i}") for i in range(2)]
            for ch in range(2):
                nc.vector.tensor_copy(p_sb[ch][:], p_ps[ch])
            pswork_cm.__exit__(None, None, None)

            # ---------- outT = P^T @ embT_chunk  (float32r) ----------
            psout_cm = tc.tile_pool(name="psout", bufs=1, space="PSUM")
            psout = psout_cm.__enter__()
            for nb in range(CHUNK // 512):
                ns = slice(nb * 512, (nb + 1) * 512)
                for ch in range(2):
                    o_ps = psout.tile([128, 512], F32, name="ops", tag="ops",
                                      bufs=4)
                    for kc in range(2):
                        nc.tensor.matmul(
                            o_ps[:],
                            p_sb[kc][:, ch * 128:(ch + 1) * 128],
                            embt_full[:, kc * CHUNK + nb * 512:
                                      kc * CHUNK + (nb + 1) * 512],
                            start=(kc == 0), stop=(kc == 1))
                    o_sb = chain.tile([128, 512], F32, name="osb", tag="osb")
                    nc.vector.tensor_copy(o_sb[:], o_ps[:])
                    nc.sync.dma_start(outt[ch * 128:(ch + 1) * 128, ns], o_sb[:])
            psout_cm.__exit__(None, None, None)

    nc.compile()
    return nc


_NC_CACHE = None


def kernel(emb, Wq, Wk, Wv, Wo):
    global _NC_CACHE
    emb = np.ascontiguousarray(np.asarray(emb, dtype=np.float32))
    Wq = np.ascontiguousarray(np.asarray(Wq, dtype=np.float32))
    Wk = np.ascontiguousarray(np.asarray(Wk, dtype=np.float32))
    Wv = np.ascontiguousarray(np.asarray(Wv, dtype=np.float32))
    Wo = np.ascontiguousarray(np.asarray(Wo, dtype=np.float32))

    if _NC_CACHE is None:
        _NC_CACHE = build_kernel()
    nc = _NC_CACHE

    # Wv^T rows are h*256+e; pack p-major: [p, t*256+c] = row t*128+p
    wvt = np.ascontiguousarray(
        Wv.T.reshape(16, 128, C).transpose(1, 0, 2).reshape(128, 16 * C))
    # Wo rows are d*H+h; regroup per head then pack p-major
    wos_n = Wo.reshape(C, H, C).transpose(1, 0, 2).reshape(C * H, C)
    wos = np.ascontiguousarray(
        wos_n.reshape(16, 128, C).transpose(1, 0, 2).reshape(128, 16 * C))
    # wqk [128, 8192]: wk half0 | wk half1 | wq half0 | wq half1
    wqk = np.ascontiguousarray(np.concatenate(
        [Wk[:128], Wk[128:], Wq[:128], Wq[128:]], axis=1))

    in_maps = []
    for c in range(NCORES):
        b, j = divmod(c, NCORES // B)
        e_b = emb[b]
        et = e_b[j * CHUNK:(j + 1) * CHUNK, :].T  # [256, 2048]
        etp = np.ascontiguousarray(np.concatenate([et[:128], et[128:]], axis=1))
        in_maps.append({
            "emb": e_b,
            "wqk": wqk, "wvt": wvt, "wos": wos,
            "embt": etp,
        })

    trace = bool(int(os.environ.get("KERNEL_TRACE", "0")))
    res = run_bass_kernel_spmd(nc, in_maps, core_ids=list(range(NCORES)),
                               trace=trace)
    kernel.last_result = res

    full = np.empty((B, N, C), dtype=np.float32)
    for c in range(NCORES):
        b, j = divmod(c, NCORES // B)
        full[b, j * CHUNK:(j + 1) * CHUNK, :] = res.results[c]["outt"].T
    return full

